# revision 13
# baseline (speedup 1.0000x reference)
"""CLIP-style contrastive (HCL) loss for B=4096, f32 logits on 8 trn2 cores.

Math reduction (BETA=1, t=0.5, tau+=0.1):
  - imp == neg, so reweight_neg = sum(neg^2) * N / sum(neg).
  - Row i and row i+B of the 2Bx2B sim matrix hold identical value multisets
    (both are {row_i(L), col_i(L)} minus two copies of L[i,i]), so
    loss[i] == loss[i+B] and the mean over 2B rows == mean over B rows.
  - Everything reduces to row sums + col sums of E = exp(2L) and E2 = exp(4L),
    plus the diagonal of L.

Device work per core (rows k*512..(k+1)*512 of L):
  - 4 tiles [128, 4096]: ACT exp(2x) and exp(4x) with fused accum_out row sums,
    PE ones-matmul per-tile column sums into PSUM [8, 4096].
Host: assemble sums, tiny per-row loss formula over 4096 rows, mean.
"""

import os

import numpy as np

import concourse.bacc as bacc
import concourse.bass as bass
import concourse.tile as tile
from concourse import mybir
from concourse.bass_utils import run_bass_kernel_spmd

B = 4096
N_CORES = 8
ROWS_PER_CORE = B // N_CORES  # 512
P = 128
TILES = ROWS_PER_CORE // P  # 4
CHUNK = 512  # matmul free-dim max for fp32 (one PSUM bank)
NCHUNK = B // CHUNK  # 8

TAU_PLUS = 0.1
TEMPERATURE = 0.5
EPS = 1e-8

_NC = None
LAST_RESULTS = None  # BassKernelResults of the most recent run (for test harness)


def _build_bass():
    nc = bacc.Bacc(None)
    slab = nc.declare_dram_parameter(
        "slab", [ROWS_PER_CORE, B], mybir.dt.float32, isOutput=False
    )
    rowsums = nc.declare_dram_parameter(
        "rowsums", [P, 2 * TILES], mybir.dt.float32, isOutput=True
    )
    colsums = nc.declare_dram_parameter(
        "colsums", [1, 2 * B], mybir.dt.float32, isOutput=True
    )

    with tile.TileContext(nc) as tc:
        with (
            tc.tile_pool(name="lpool", bufs=2) as lpool,
            tc.tile_pool(name="epool", bufs=2) as epool,
            tc.tile_pool(name="e2pool", bufs=2) as e2pool,
            tc.tile_pool(name="singles", bufs=1) as singles,
            tc.tile_pool(name="psum", bufs=1, space="PSUM") as psum_pool,
        ):
            ones = nc.const_aps.tensor(1.0, (P, 1), mybir.dt.float32)
            rs = singles.tile([P, 2 * TILES], mybir.dt.float32)
            # row 0: colsum(E) accumulator; row 32: colsum(E2). Matmul output
            # base partition must be one of {0, 32, 64}.
            ps = psum_pool.tile([33, B], mybir.dt.float32)

            for t in range(TILES):
                ltile = lpool.tile([P, B], mybir.dt.float32)
                nc.sync.dma_start(out=ltile, in_=slab[t * P : (t + 1) * P, :])

                etile = epool.tile([P, B], mybir.dt.float32)
                nc.scalar.activation(
                    out=etile,
                    in_=ltile,
                    func=mybir.ActivationFunctionType.Exp,
                    scale=2.0,
                    accum_out=rs[:, t : t + 1],
                )
                e2tile = e2pool.tile([P, B], mybir.dt.float32)
                nc.scalar.activation(
                    out=e2tile,
                    in_=ltile,
                    func=mybir.ActivationFunctionType.Exp,
                    scale=4.0,
                    accum_out=rs[:, TILES + t : TILES + t + 1],
                )

                for c in range(NCHUNK):
                    sl = slice(c * CHUNK, (c + 1) * CHUNK)
                    nc.tensor.matmul(
                        ps[0:1, sl],
                        ones,
                        etile[:, sl],
                        start=(t == 0),
                        stop=(t == TILES - 1),
                    )
                    nc.tensor.matmul(
                        ps[32:33, sl],
                        ones,
                        e2tile[:, sl],
                        start=(t == 0),
                        stop=(t == TILES - 1),
                    )

            cs = singles.tile([1, 2 * B], mybir.dt.float32)
            nc.vector.tensor_copy(cs[:, 0:B], ps[0:1, :])
            nc.scalar.copy(cs[:, B : 2 * B], ps[32:33, :])
            nc.sync.dma_start(out=colsums[:, :], in_=cs)
            nc.sync.dma_start(out=rowsums[:, :], in_=rs)
    # Bacc defers register allocation and sync-wait splitting to finalize();
    # run_bass_via_pjrt does not call it, so do it here.
    nc.finalize()
    return nc


def _get_nc():
    global _NC
    if _NC is None:
        _NC = _build_bass()
    return _NC


def kernel(logits: np.ndarray) -> np.ndarray:
    global LAST_RESULTS
    logits = np.ascontiguousarray(np.asarray(logits, dtype=np.float32))
    assert logits.shape == (B, B)

    nc = _get_nc()
    in_maps = [
        {"slab": np.ascontiguousarray(logits[k * ROWS_PER_CORE : (k + 1) * ROWS_PER_CORE, :])}
        for k in range(N_CORES)
    ]
    res = run_bass_kernel_spmd(
        nc,
        in_maps,
        core_ids=list(range(N_CORES)),
        trace=bool(int(os.environ.get("KERNEL_TRACE", "0"))),
    )
    LAST_RESULTS = res

    rowsum_E = np.empty(B, dtype=np.float64)
    rowsum_E2 = np.empty(B, dtype=np.float64)
    colsum_E = np.zeros(B, dtype=np.float64)
    colsum_E2 = np.zeros(B, dtype=np.float64)
    for k in range(N_CORES):
        r = res.results[k]
        rs = r["rowsums"].astype(np.float64)  # [128, 8]
        sl = slice(k * ROWS_PER_CORE, (k + 1) * ROWS_PER_CORE)
        rowsum_E[sl] = rs[:, :TILES].T.reshape(-1)
        rowsum_E2[sl] = rs[:, TILES:].T.reshape(-1)
        cssum = r["colsums"].astype(np.float64).reshape(2, B)
        colsum_E += cssum[0]
        colsum_E2 += cssum[1]

    d = np.diagonal(logits).astype(np.float64)
    pos = np.exp(d / TEMPERATURE)
    N = 2 * B - 2
    S1 = rowsum_E + colsum_E - 2.0 * pos
    S2 = rowsum_E2 + colsum_E2 - 2.0 * pos * pos
    reweight = S2 * N / S1
    Ng = (-TAU_PLUS * N * pos + reweight) / (1.0 - TAU_PLUS)
    Ng = np.maximum(Ng, N * np.exp(-1.0 / TEMPERATURE))
    loss = -np.log(pos / (pos + Ng + EPS))
    return np.float32(loss.mean())


# revision 18
# speedup vs baseline: 1.0923x; 1.0923x over previous
"""CLIP-style contrastive (HCL) loss for B=4096, f32 logits on 8 trn2 cores.

Math reduction (BETA=1, t=0.5, tau+=0.1):
  - imp == neg, so reweight_neg = sum(neg^2) * N / sum(neg).
  - Row i and row i+B of the 2Bx2B sim matrix hold identical value multisets
    (both are {row_i(L), col_i(L)} minus two copies of L[i,i]), so
    loss[i] == loss[i+B] and the mean over 2B rows == mean over B rows.
  - Everything reduces to row sums + col sums of E = exp(2L) and E2 = exp(4L),
    plus the diagonal of L.

Device work per core (rows k*512..(k+1)*512 of L):
  - 4 tiles [128, 4096]: ACT exp(2x) and exp(4x) with fused accum_out row sums,
    PE ones-matmul per-tile column sums into PSUM [8, 4096].
Host: assemble sums, tiny per-row loss formula over 4096 rows, mean.
"""

import os

import numpy as np

import concourse.bacc as bacc
import concourse.bass as bass
import concourse.tile as tile
from concourse import mybir
from concourse.bass_utils import run_bass_kernel_spmd

B = 4096
N_CORES = 8
ROWS_PER_CORE = B // N_CORES  # 512
P = 128
TILES = ROWS_PER_CORE // P  # 4
CHUNK = 512  # matmul free-dim max for fp32 (one PSUM bank)
NCHUNK = B // CHUNK  # 8

TAU_PLUS = 0.1
TEMPERATURE = 0.5
EPS = 1e-8

_NC = None
LAST_RESULTS = None  # BassKernelResults of the most recent run (for test harness)


HALF = B // 2  # 2048 cols per half-tile
NHALF = 2 * TILES  # 8 half-tiles per core
USE_BF16 = bool(int(os.environ.get("KERNEL_BF16", "1")))
USE_TTR = bool(int(os.environ.get("KERNEL_TTR", "1")))


def _build_bass():
    edt = mybir.dt.bfloat16 if USE_BF16 else mybir.dt.float32
    nc = bacc.Bacc(None)
    slab = nc.declare_dram_parameter(
        "slab", [ROWS_PER_CORE, B], mybir.dt.float32, isOutput=False
    )
    rowsums = nc.declare_dram_parameter(
        "rowsums", [P, 2 * NHALF], mybir.dt.float32, isOutput=True
    )
    colsums = nc.declare_dram_parameter(
        "colsums", [1, 2 * B], mybir.dt.float32, isOutput=True
    )

    with tile.TileContext(nc) as tc:
        with (
            tc.tile_pool(name="lpool", bufs=6) as lpool,
            tc.tile_pool(name="epool", bufs=3) as epool,
            tc.tile_pool(name="e2pool", bufs=3) as e2pool,
            tc.tile_pool(name="singles", bufs=1) as singles,
            tc.tile_pool(name="psum", bufs=1, space="PSUM") as psum_pool,
        ):
            ones = nc.const_aps.tensor(1.0, (P, 1), edt)
            rs = singles.tile([P, 2 * NHALF], mybir.dt.float32)
            # row 0: colsum(E) accumulator; row 32: colsum(E2). Matmul output
            # base partition must be one of {0, 32, 64}.
            ps = psum_pool.tile([33, B], mybir.dt.float32)

            for h in range(NHALF):
                t, half = divmod(h, 2)
                rows = slice(t * P, (t + 1) * P)
                cols = slice(half * HALF, (half + 1) * HALF)

                ltile = lpool.tile([P, HALF], mybir.dt.float32)
                nc.sync.dma_start(out=ltile, in_=slab[rows, cols])

                etile = epool.tile([P, HALF], edt)
                nc.scalar.activation(
                    out=etile,
                    in_=ltile,
                    func=mybir.ActivationFunctionType.Exp,
                    scale=2.0,
                    accum_out=rs[:, h : h + 1],
                )
                e2tile = e2pool.tile([P, HALF], edt)
                if USE_TTR:
                    nc.vector.tensor_tensor_reduce(
                        out=e2tile,
                        in0=etile,
                        in1=etile,
                        scale=1.0,
                        scalar=0.0,
                        op0=mybir.AluOpType.mult,
                        op1=mybir.AluOpType.add,
                        accum_out=rs[:, NHALF + h : NHALF + h + 1],
                    )
                else:
                    nc.scalar.activation(
                        out=e2tile,
                        in_=ltile,
                        func=mybir.ActivationFunctionType.Exp,
                        scale=4.0,
                        accum_out=rs[:, NHALF + h : NHALF + h + 1],
                    )

                for c in range(HALF // CHUNK):
                    lsl = slice(c * CHUNK, (c + 1) * CHUNK)
                    gsl = slice(half * HALF + c * CHUNK, half * HALF + (c + 1) * CHUNK)
                    nc.tensor.matmul(
                        ps[0:1, gsl],
                        ones,
                        etile[:, lsl],
                        start=(t == 0),
                        stop=(t == TILES - 1),
                    )
                    nc.tensor.matmul(
                        ps[32:33, gsl],
                        ones,
                        e2tile[:, lsl],
                        start=(t == 0),
                        stop=(t == TILES - 1),
                    )

            cs = singles.tile([1, 2 * B], mybir.dt.float32)
            nc.vector.tensor_copy(cs[:, 0:B], ps[0:1, :])
            nc.scalar.copy(cs[:, B : 2 * B], ps[32:33, :])
            nc.sync.dma_start(out=colsums[:, :], in_=cs)
            nc.sync.dma_start(out=rowsums[:, :], in_=rs)
    # Bacc defers register allocation and sync-wait splitting to finalize();
    # run_bass_via_pjrt does not call it, so do it here.
    nc.finalize()
    return nc


def _get_nc():
    global _NC
    if _NC is None:
        _NC = _build_bass()
    return _NC


def kernel(logits: np.ndarray) -> np.ndarray:
    global LAST_RESULTS
    logits = np.ascontiguousarray(np.asarray(logits, dtype=np.float32))
    assert logits.shape == (B, B)

    nc = _get_nc()
    in_maps = [
        {"slab": np.ascontiguousarray(logits[k * ROWS_PER_CORE : (k + 1) * ROWS_PER_CORE, :])}
        for k in range(N_CORES)
    ]
    res = run_bass_kernel_spmd(
        nc,
        in_maps,
        core_ids=list(range(N_CORES)),
        trace=bool(int(os.environ.get("KERNEL_TRACE", "0"))),
    )
    LAST_RESULTS = res

    rowsum_E = np.empty(B, dtype=np.float64)
    rowsum_E2 = np.empty(B, dtype=np.float64)
    colsum_E = np.zeros(B, dtype=np.float64)
    colsum_E2 = np.zeros(B, dtype=np.float64)
    for k in range(N_CORES):
        r = res.results[k]
        rs = r["rowsums"].astype(np.float64)  # [128, 16]: half-tile partials
        sl = slice(k * ROWS_PER_CORE, (k + 1) * ROWS_PER_CORE)
        rsE = rs[:, :NHALF].reshape(P, TILES, 2).sum(axis=2)  # [128, 4]
        rsE2 = rs[:, NHALF:].reshape(P, TILES, 2).sum(axis=2)
        rowsum_E[sl] = rsE.T.reshape(-1)
        rowsum_E2[sl] = rsE2.T.reshape(-1)
        cssum = r["colsums"].astype(np.float64).reshape(2, B)
        colsum_E += cssum[0]
        colsum_E2 += cssum[1]

    d = np.diagonal(logits).astype(np.float64)
    pos = np.exp(d / TEMPERATURE)
    N = 2 * B - 2
    S1 = rowsum_E + colsum_E - 2.0 * pos
    S2 = rowsum_E2 + colsum_E2 - 2.0 * pos * pos
    reweight = S2 * N / S1
    Ng = (-TAU_PLUS * N * pos + reweight) / (1.0 - TAU_PLUS)
    Ng = np.maximum(Ng, N * np.exp(-1.0 / TEMPERATURE))
    loss = -np.log(pos / (pos + Ng + EPS))
    return np.float32(loss.mean())


# revision 20
# speedup vs baseline: 1.4613x; 1.3379x over previous
"""CLIP-style contrastive (HCL) loss for B=4096, f32 logits on 8 trn2 cores.

Math reduction (BETA=1, t=0.5, tau+=0.1):
  - imp == neg, so reweight_neg = sum(neg^2) * N / sum(neg).
  - Row i and row i+B of the 2Bx2B sim matrix hold identical value multisets
    (both are {row_i(L), col_i(L)} minus two copies of L[i,i]), so
    loss[i] == loss[i+B] and the mean over 2B rows == mean over B rows.
  - Everything reduces to row sums + col sums of E = exp(2L) and E2 = exp(4L),
    plus the diagonal of L.

Device work per core (rows k*512..(k+1)*512 of L, cast to bf16 on host):
  - 8 half-tiles [128, 2048]: ACT exp(2x)->bf16 E with fused fp32 row-sum,
    DVE scalar_tensor_tensor E*E->bf16 E2 with fused fp32 row-sum,
    PE ones-matmul per-column sums into PSUM (E at partition 0, E2 at 32).
Host: assemble sums, per-row loss formula over 4096 rows in f64, mean.
"""

import os

import numpy as np
import ml_dtypes

import concourse.bacc as bacc
import concourse.bass as bass
import concourse.tile as tile
from concourse import mybir
from concourse.bass_utils import run_bass_kernel_spmd

B = 4096
N_CORES = 8
ROWS_PER_CORE = B // N_CORES  # 512
P = 128
TILES = ROWS_PER_CORE // P  # 4
CHUNK = 512  # matmul free-dim max (one PSUM bank)
HALF = B // 2  # 2048 cols per half-tile
NHALF = 2 * TILES  # 8 half-tiles per core

TAU_PLUS = 0.1
TEMPERATURE = 0.5
EPS = 1e-8

USE_BF16_IN = bool(int(os.environ.get("KERNEL_BF16_IN", "1")))
USE_STT = bool(int(os.environ.get("KERNEL_STT", "1")))

_NC = None
LAST_RESULTS = None  # BassKernelResults of the most recent run (for test harness)


def _build_bass():
    in_dt = mybir.dt.bfloat16 if USE_BF16_IN else mybir.dt.float32
    edt = mybir.dt.bfloat16

    nc = bacc.Bacc(None)
    slab = nc.declare_dram_parameter("slab", [ROWS_PER_CORE, B], in_dt, isOutput=False)
    rowsums = nc.declare_dram_parameter(
        "rowsums", [P, 2 * NHALF], mybir.dt.float32, isOutput=True
    )
    colsums = nc.declare_dram_parameter(
        "colsums", [2, B], mybir.dt.float32, isOutput=True
    )

    with tile.TileContext(nc) as tc:
        with (
            tc.tile_pool(name="lpool", bufs=6) as lpool,
            tc.tile_pool(name="epool", bufs=3) as epool,
            tc.tile_pool(name="e2pool", bufs=3) as e2pool,
            tc.tile_pool(name="singles", bufs=1) as singles,
            tc.tile_pool(name="psum", bufs=1, space="PSUM") as psum_pool,
        ):
            ones = nc.const_aps.tensor(1.0, (P, 1), mybir.dt.bfloat16)
            rs = singles.tile([P, 2 * NHALF], mybir.dt.float32)
            # Colsum accumulators: E at partition 0, E2 at partition 32 (matmul
            # output base partition must be one of {0, 32, 64}).
            ps = psum_pool.tile([33, B], mybir.dt.float32)

            for h in range(NHALF):
                t, half = divmod(h, 2)
                rows = slice(t * P, (t + 1) * P)
                cols = slice(half * HALF, (half + 1) * HALF)

                ltile = lpool.tile([P, HALF], in_dt)
                nc.sync.dma_start(out=ltile, in_=slab[rows, cols])

                etile = epool.tile([P, HALF], edt)
                nc.scalar.activation(
                    out=etile,
                    in_=ltile,
                    func=mybir.ActivationFunctionType.Exp,
                    scale=2.0,
                    accum_out=rs[:, h : h + 1],
                )
                e2tile = e2pool.tile([P, HALF], edt)
                if USE_STT:
                    # E2 = (E * 1) * E on DVE, with fused fp32 row-sum.
                    nc.vector.scalar_tensor_tensor(
                        out=e2tile,
                        in0=etile,
                        scalar=1.0,
                        in1=etile,
                        op0=mybir.AluOpType.mult,
                        op1=mybir.AluOpType.mult,
                        accum_out=rs[:, NHALF + h : NHALF + h + 1],
                    )
                else:
                    nc.scalar.activation(
                        out=e2tile,
                        in_=ltile,
                        func=mybir.ActivationFunctionType.Exp,
                        scale=4.0,
                        accum_out=rs[:, NHALF + h : NHALF + h + 1],
                    )

                for c in range(HALF // CHUNK):
                    lsl = slice(c * CHUNK, (c + 1) * CHUNK)
                    gsl = slice(half * HALF + c * CHUNK, half * HALF + (c + 1) * CHUNK)
                    nc.tensor.matmul(
                        ps[0:1, gsl],
                        ones,
                        etile[:, lsl],
                        start=(t == 0),
                        stop=(t == TILES - 1),
                    )
                    nc.tensor.matmul(
                        ps[32:33, gsl],
                        ones,
                        e2tile[:, lsl],
                        start=(t == 0),
                        stop=(t == TILES - 1),
                    )

            # Concurrent eviction: separate SBUF tiles so the two engines don't
            # serialize on a shared-output WAW hazard.
            csE = singles.tile([1, B], mybir.dt.float32)
            csE2 = singles.tile([1, B], mybir.dt.float32)
            nc.vector.tensor_copy(csE, ps[0:1, :])
            nc.scalar.copy(csE2, ps[32:33, :])
            nc.sync.dma_start(out=colsums[0:1, :], in_=csE)
            nc.sync.dma_start(out=colsums[1:2, :], in_=csE2)
            nc.sync.dma_start(out=rowsums[:, :], in_=rs)
    # Bacc defers register allocation and sync-wait splitting to finalize();
    # run_bass_via_pjrt does not call it, so do it here.
    nc.finalize()
    return nc


def _get_nc():
    global _NC
    if _NC is None:
        _NC = _build_bass()
    return _NC


def kernel(logits: np.ndarray) -> np.ndarray:
    global LAST_RESULTS
    logits = np.ascontiguousarray(np.asarray(logits, dtype=np.float32))
    assert logits.shape == (B, B)

    nc = _get_nc()
    if USE_BF16_IN:
        cast = lambda a: np.ascontiguousarray(a.astype(ml_dtypes.bfloat16))
    else:
        cast = np.ascontiguousarray
    in_maps = [
        {"slab": cast(logits[k * ROWS_PER_CORE : (k + 1) * ROWS_PER_CORE, :])}
        for k in range(N_CORES)
    ]
    res = run_bass_kernel_spmd(
        nc,
        in_maps,
        core_ids=list(range(N_CORES)),
        trace=bool(int(os.environ.get("KERNEL_TRACE", "0"))),
    )
    LAST_RESULTS = res

    rowsum_E = np.empty(B, dtype=np.float64)
    rowsum_E2 = np.empty(B, dtype=np.float64)
    colsum_E = np.zeros(B, dtype=np.float64)
    colsum_E2 = np.zeros(B, dtype=np.float64)
    for k in range(N_CORES):
        r = res.results[k]
        rs = r["rowsums"].astype(np.float64)  # [128, 16]: half-tile partials
        sl = slice(k * ROWS_PER_CORE, (k + 1) * ROWS_PER_CORE)
        rsE = rs[:, :NHALF].reshape(P, TILES, 2).sum(axis=2)  # [128, 4]
        rsE2 = rs[:, NHALF:].reshape(P, TILES, 2).sum(axis=2)
        rowsum_E[sl] = rsE.T.reshape(-1)
        rowsum_E2[sl] = rsE2.T.reshape(-1)
        cssum = r["colsums"].astype(np.float64)  # [2, 4096]
        colsum_E += cssum[0]
        colsum_E2 += cssum[1]

    d = np.diagonal(logits)
    pos = np.exp(d.astype(np.float64) / TEMPERATURE)
    if USE_BF16_IN:
        # The device sums contain exp of the bf16-rounded diagonal; subtract
        # exactly what the device added.
        dD = d.astype(ml_dtypes.bfloat16).astype(np.float64)
    else:
        dD = d.astype(np.float64)
    posD = np.exp(dD / TEMPERATURE)
    N = 2 * B - 2
    S1 = rowsum_E + colsum_E - 2.0 * posD
    S2 = rowsum_E2 + colsum_E2 - 2.0 * posD * posD
    reweight = S2 * N / S1
    Ng = (-TAU_PLUS * N * pos + reweight) / (1.0 - TAU_PLUS)
    Ng = np.maximum(Ng, N * np.exp(-1.0 / TEMPERATURE))
    loss = -np.log(pos / (pos + Ng + EPS))
    return np.float32(loss.mean())


# revision 26
# speedup vs baseline: 1.7705x; 1.2116x over previous
"""CLIP-style contrastive (HCL) loss for B=4096, f32 logits on 8 trn2 cores.

Math reduction (BETA=1, t=0.5, tau+=0.1):
  - imp == neg, so reweight_neg = sum(neg^2) * N / sum(neg).
  - Row i and row i+B of the 2Bx2B sim matrix hold identical value multisets
    (both are {row_i(L), col_i(L)} minus two copies of L[i,i]), so
    loss[i] == loss[i+B] and the mean over 2B rows == mean over B rows.
  - Everything reduces to row sums + col sums of E = exp(2L) and E2 = exp(4L),
    plus the diagonal of L.

Device work per core (rows k*512..(k+1)*512 of L, cast to bf16 on host):
  - 8 half-tiles [128, 2048]: ACT exp(2x)->bf16 E with fused fp32 row-sum,
    DVE scalar_tensor_tensor E*E->bf16 E2 with fused fp32 row-sum,
    PE ones-matmul per-column sums into PSUM (E at partition 0, E2 at 32).
Host: assemble sums, per-row loss formula over 4096 rows in f64, mean.
"""

import os

import numpy as np
import ml_dtypes

import concourse.bacc as bacc
import concourse.bass as bass
import concourse.tile as tile
from concourse import mybir
from concourse.bass_utils import run_bass_kernel_spmd

B = 4096
N_CORES = 8
ROWS_PER_CORE = B // N_CORES  # 512
P = 128
TILES = ROWS_PER_CORE // P  # 4
CHUNK = 512  # matmul free-dim max (one PSUM bank)
HALF = B // 2  # 2048 cols per half-tile
NHALF = 2 * TILES  # 8 half-tiles per core

TAU_PLUS = 0.1
TEMPERATURE = 0.5
EPS = 1e-8

USE_BF16_IN = bool(int(os.environ.get("KERNEL_BF16_IN", "1")))
USE_STT = bool(int(os.environ.get("KERNEL_STT", "1")))
USE_CSTAT = bool(int(os.environ.get("KERNEL_CSTAT", "1")))  # chunk-stationary colsums
# GpSimd (Pool) rejects TensorScalarPtr at codegen ISA check; keep at 0.
N_GPSIMD_SQ = int(os.environ.get("KERNEL_GP_SQ", "0"))
LPOOL_BUFS = int(os.environ.get("KERNEL_LPOOL_BUFS", "3"))

_NC = None
LAST_RESULTS = None  # BassKernelResults of the most recent run (for test harness)


def _build_bass():
    in_dt = mybir.dt.bfloat16 if USE_BF16_IN else mybir.dt.float32
    edt = mybir.dt.bfloat16

    nc = bacc.Bacc(None)
    slab = nc.declare_dram_parameter("slab", [ROWS_PER_CORE, B], in_dt, isOutput=False)
    rowsums = nc.declare_dram_parameter(
        "rowsums", [P, 2 * NHALF], mybir.dt.float32, isOutput=True
    )
    # Chunk-stationary layout: [128, 64] (E cols 0:32, E2 cols 32:64), where
    # colsum[m*128 + j] = out[j, m]. Row layout: [2, B].
    cshape = [P, 2 * (B // P)] if USE_CSTAT else [2, B]
    colsums = nc.declare_dram_parameter(
        "colsums", cshape, mybir.dt.float32, isOutput=True
    )

    with tile.TileContext(nc) as tc:
        with (
            tc.tile_pool(name="lpool", bufs=LPOOL_BUFS) as lpool,
            tc.tile_pool(name="epool", bufs=3) as epool,
            tc.tile_pool(name="e2pool", bufs=3) as e2pool,
            tc.tile_pool(name="singles", bufs=1) as singles,
            tc.tile_pool(name="psum", bufs=1, space="PSUM") as psum_pool,
        ):
            ones = nc.const_aps.tensor(1.0, (P, 1), mybir.dt.bfloat16)
            rs = singles.tile([P, 2 * NHALF], mybir.dt.float32)
            if USE_CSTAT:
                # One PSUM bank per accumulator; output [128, 32] each.
                psE = psum_pool.tile([P, B // P], mybir.dt.float32)
                psE2 = psum_pool.tile([P, B // P], mybir.dt.float32)
            else:
                # Colsum accumulators: E at partition 0, E2 at partition 32
                # (matmul output base partition must be one of {0, 32, 64}).
                ps = psum_pool.tile([33, B], mybir.dt.float32)

            for h in range(NHALF):
                t, half = divmod(h, 2)
                rows = slice(t * P, (t + 1) * P)
                cols = slice(half * HALF, (half + 1) * HALF)

                ltile = lpool.tile([P, HALF], in_dt)
                nc.sync.dma_start(out=ltile, in_=slab[rows, cols])

                etile = epool.tile([P, HALF], edt)
                nc.scalar.activation(
                    out=etile,
                    in_=ltile,
                    func=mybir.ActivationFunctionType.Exp,
                    scale=2.0,
                    accum_out=rs[:, h : h + 1],
                )
                e2tile = e2pool.tile([P, HALF], edt)
                # The last N_GPSIMD_SQ squares go to GpSimd to unload DVE.
                sq_engine = (
                    nc.gpsimd if (USE_STT and h >= NHALF - N_GPSIMD_SQ) else nc.vector
                )
                if USE_STT:
                    # E2 = (E * 1) * E, with fused fp32 row-sum.
                    sq_engine.scalar_tensor_tensor(
                        out=e2tile,
                        in0=etile,
                        scalar=1.0,
                        in1=etile,
                        op0=mybir.AluOpType.mult,
                        op1=mybir.AluOpType.mult,
                        accum_out=rs[:, NHALF + h : NHALF + h + 1],
                    )
                else:
                    nc.scalar.activation(
                        out=e2tile,
                        in_=ltile,
                        func=mybir.ActivationFunctionType.Exp,
                        scale=4.0,
                        accum_out=rs[:, NHALF + h : NHALF + h + 1],
                    )

                if USE_CSTAT:
                    for m in range(HALF // P):
                        gm = half * (HALF // P) + m
                        lsl = slice(m * P, (m + 1) * P)
                        nc.tensor.matmul(
                            psE[:, gm : gm + 1],
                            etile[:, lsl],
                            ones,
                            start=(t == 0),
                            stop=(t == TILES - 1),
                        )
                        nc.tensor.matmul(
                            psE2[:, gm : gm + 1],
                            e2tile[:, lsl],
                            ones,
                            start=(t == 0),
                            stop=(t == TILES - 1),
                        )
                else:
                    for c in range(HALF // CHUNK):
                        lsl = slice(c * CHUNK, (c + 1) * CHUNK)
                        gsl = slice(
                            half * HALF + c * CHUNK, half * HALF + (c + 1) * CHUNK
                        )
                        nc.tensor.matmul(
                            ps[0:1, gsl],
                            ones,
                            etile[:, lsl],
                            start=(t == 0),
                            stop=(t == TILES - 1),
                        )
                        nc.tensor.matmul(
                            ps[32:33, gsl],
                            ones,
                            e2tile[:, lsl],
                            start=(t == 0),
                            stop=(t == TILES - 1),
                        )

            if USE_CSTAT:
                M = B // P  # 32
                cs = singles.tile([P, 2 * M], mybir.dt.float32)
                nc.vector.tensor_copy(cs[:, 0:M], psE)
                nc.scalar.copy(cs[:, M : 2 * M], psE2)
                nc.sync.dma_start(out=colsums[:, :], in_=cs)
            else:
                # Concurrent eviction: separate SBUF tiles so the two engines
                # don't serialize on a shared-output WAW hazard.
                csE = singles.tile([1, B], mybir.dt.float32)
                csE2 = singles.tile([1, B], mybir.dt.float32)
                nc.vector.tensor_copy(csE, ps[0:1, :])
                nc.scalar.copy(csE2, ps[32:33, :])
                nc.sync.dma_start(out=colsums[0:1, :], in_=csE)
                nc.sync.dma_start(out=colsums[1:2, :], in_=csE2)
            nc.sync.dma_start(out=rowsums[:, :], in_=rs)
    # Bacc defers register allocation and sync-wait splitting to finalize();
    # run_bass_via_pjrt does not call it, so do it here.
    nc.finalize()
    return nc


def _get_nc():
    global _NC
    if _NC is None:
        _NC = _build_bass()
    return _NC


def kernel(logits: np.ndarray) -> np.ndarray:
    global LAST_RESULTS
    logits = np.ascontiguousarray(np.asarray(logits, dtype=np.float32))
    assert logits.shape == (B, B)

    nc = _get_nc()
    if USE_BF16_IN:
        cast = lambda a: np.ascontiguousarray(a.astype(ml_dtypes.bfloat16))
    else:
        cast = np.ascontiguousarray
    in_maps = [
        {"slab": cast(logits[k * ROWS_PER_CORE : (k + 1) * ROWS_PER_CORE, :])}
        for k in range(N_CORES)
    ]
    res = run_bass_kernel_spmd(
        nc,
        in_maps,
        core_ids=list(range(N_CORES)),
        trace=bool(int(os.environ.get("KERNEL_TRACE", "0"))),
    )
    LAST_RESULTS = res

    rowsum_E = np.empty(B, dtype=np.float64)
    rowsum_E2 = np.empty(B, dtype=np.float64)
    colsum_E = np.zeros(B, dtype=np.float64)
    colsum_E2 = np.zeros(B, dtype=np.float64)
    for k in range(N_CORES):
        r = res.results[k]
        rs = r["rowsums"].astype(np.float64)  # [128, 16]: half-tile partials
        sl = slice(k * ROWS_PER_CORE, (k + 1) * ROWS_PER_CORE)
        rsE = rs[:, :NHALF].reshape(P, TILES, 2).sum(axis=2)  # [128, 4]
        rsE2 = rs[:, NHALF:].reshape(P, TILES, 2).sum(axis=2)
        rowsum_E[sl] = rsE.T.reshape(-1)
        rowsum_E2[sl] = rsE2.T.reshape(-1)
        cssum = r["colsums"].astype(np.float64)
        if USE_CSTAT:
            M = B // P
            colsum_E += cssum[:, :M].T.reshape(-1)
            colsum_E2 += cssum[:, M:].T.reshape(-1)
        else:
            colsum_E += cssum[0]
            colsum_E2 += cssum[1]

    d = np.diagonal(logits)
    pos = np.exp(d.astype(np.float64) / TEMPERATURE)
    if USE_BF16_IN:
        # The device sums contain exp of the bf16-rounded diagonal; subtract
        # exactly what the device added.
        dD = d.astype(ml_dtypes.bfloat16).astype(np.float64)
    else:
        dD = d.astype(np.float64)
    posD = np.exp(dD / TEMPERATURE)
    N = 2 * B - 2
    S1 = rowsum_E + colsum_E - 2.0 * posD
    S2 = rowsum_E2 + colsum_E2 - 2.0 * posD * posD
    reweight = S2 * N / S1
    Ng = (-TAU_PLUS * N * pos + reweight) / (1.0 - TAU_PLUS)
    Ng = np.maximum(Ng, N * np.exp(-1.0 / TEMPERATURE))
    loss = -np.log(pos / (pos + Ng + EPS))
    return np.float32(loss.mean())
